# revision 1
# baseline (speedup 1.0000x reference)
"""Trainium2 Bass kernel for nn_GNN_EBM (gnn_message_passing).

Math: the reference broadcasts one shared feature vector h0[b,:] to all
d_nodes graph nodes before message passing, and the adjacency
A = sigmoid(B_param) * mask is elementwise non-negative.  Hence

  conv1:  relu(h0*(1 + rowsum(A)_i/N))      = c_i * relu(h0)   (c_i > 0)
  conv2:  relu(r*(c_i + (A@c)_i/N))         = g_i * r          (r >= 0, g_i > 0)

so the whole GNN collapses to e = MLP_T(g_T * r) + MLP_Y(g_Y * r) with
r = relu(z @ fc_in_w.T + fc_in_b), and the scalars g_T, g_Y fold into the
MLP first-layer weights.  The device kernel is a fused 3-layer MLP over the
batch, data-parallel across 8 cores (256 rows/core), with activations kept
transposed ([feature, batch]) so matmul outputs chain without transposes
and biases ride the per-partition bias port of the scalar engine.
"""

import sys

sys.path.insert(0, "/opt/trn_rl_repo")

import numpy as np

import concourse.bacc as bacc
import concourse.mybir as mybir
import concourse.tile as tile
from concourse.bass_utils import run_bass_kernel_spmd


def _ensure_ntff_hook():
    # bass_utils' trace path imports antenv.axon_hooks, which some agent
    # images lack; register the ctypes-based hook ourselves so BASS_TRACE=1
    # yields an NTFF profile instead of an ImportError.
    try:
        import antenv.axon_hooks  # noqa: F401
        return
    except ImportError:
        pass
    import types

    import antenv

    mod = types.ModuleType("antenv.axon_hooks")
    holder = {"hook": None}
    mod.set_axon_ntff_profile_hook = lambda h: holder.__setitem__("hook", h)
    mod.get_axon_ntff_profile_hook = lambda: holder["hook"]
    sys.modules["antenv.axon_hooks"] = mod
    antenv.axon_hooks = mod
    try:
        from trn_agent_boot.trn_boot import _ntff_profile_via_ctypes

        hook = _ntff_profile_via_ctypes("/opt/axon/libaxon_pjrt.so")
        if hook is not None:
            mod.set_axon_ntff_profile_hook(hook)
    except Exception:
        pass


_ensure_ntff_hook()

N_CORES = 8
BATCH = 2048
D_X = 100
D_NODES = D_X + 2          # 102
D_IN = D_X + 2             # x + t + y = 102
HID = 256
MLP_HID = 128
SHARD = BATCH // N_CORES   # 256
KPAD = 128                 # input contraction dim padded 102 -> 128

F32 = mybir.dt.float32

_NC_CACHE = None
LAST_RESULT = None         # BassKernelResults of the most recent run


def _build_nc():
    nc = bacc.Bacc("TRN2", target_bir_lowering=False, debug=False,
                   num_devices=N_CORES)

    zT = nc.dram_tensor("zT", [KPAD, SHARD], F32, kind="ExternalInput")
    w_in = nc.dram_tensor("w_in", [KPAD, HID], F32, kind="ExternalInput")
    w1g = nc.dram_tensor("w1g", [2, 128, 2 * MLP_HID], F32, kind="ExternalInput")
    b_in = nc.dram_tensor("b_in", [128, 2], F32, kind="ExternalInput")
    b1 = nc.dram_tensor("b1", [128, 3], F32, kind="ExternalInput")
    w2 = nc.dram_tensor("w2", [128, 2], F32, kind="ExternalInput")
    out = nc.dram_tensor("out", [1, SHARD], F32, kind="ExternalOutput")

    RELU = mybir.ActivationFunctionType.Relu
    IDENT = mybir.ActivationFunctionType.Identity

    with tile.TileContext(nc) as tc:
        with (
            tc.tile_pool(name="sb", bufs=1) as sb,
            tc.tile_pool(name="ps", bufs=1, space="PSUM") as ps,
        ):
            zT_sb = sb.tile([KPAD, SHARD], F32, tag="zT")
            w_in_sb = sb.tile([KPAD, HID], F32, tag="w_in")
            w1_sb = sb.tile([128, 512], F32, tag="w1")
            b_in_sb = sb.tile([128, 2], F32, tag="b_in")
            b1_sb = sb.tile([128, 3], F32, tag="b1")
            w2_sb = sb.tile([128, 2], F32, tag="w2")

            nc.sync.dma_start(zT_sb[:], zT[:])
            nc.sync.dma_start(w_in_sb[:], w_in[:])
            nc.sync.dma_start(w1_sb[:, 0:256], w1g[0])
            nc.sync.dma_start(w1_sb[:, 256:512], w1g[1])
            nc.sync.dma_start(b_in_sb[:], b_in[:])
            nc.sync.dma_start(b1_sb[:], b1[:])
            nc.sync.dma_start(w2_sb[:], w2[:])

            # h0^T = fc_in_w @ z^T, feature chunks of 128 on partitions
            h_p0 = ps.tile([128, SHARD], F32, tag="h0")
            h_p1 = ps.tile([128, SHARD], F32, tag="h1")
            nc.tensor.matmul(h_p0[:], w_in_sb[:, 0:128], zT_sb[:])
            nc.tensor.matmul(h_p1[:], w_in_sb[:, 128:256], zT_sb[:])

            # r = relu(h0 + fc_in_b), still transposed
            r0 = sb.tile([128, SHARD], F32, tag="r0")
            r1 = sb.tile([128, SHARD], F32, tag="r1")
            nc.scalar.activation(r0[:], h_p0[:], RELU, bias=b_in_sb[:, 0:1])
            nc.scalar.activation(r1[:], h_p1[:], RELU, bias=b_in_sb[:, 1:2])

            # u_head^T = (g_head * w1_head) @ r^T, two k-chunks accumulated
            u_pT = ps.tile([128, SHARD], F32, tag="uT")
            u_pY = ps.tile([128, SHARD], F32, tag="uY")
            nc.tensor.matmul(u_pT[:], w1_sb[:, 0:128], r0[:], start=True, stop=False)
            nc.tensor.matmul(u_pT[:], w1_sb[:, 256:384], r1[:], start=False, stop=True)
            nc.tensor.matmul(u_pY[:], w1_sb[:, 128:256], r0[:], start=True, stop=False)
            nc.tensor.matmul(u_pY[:], w1_sb[:, 384:512], r1[:], start=False, stop=True)

            uT_sb = sb.tile([128, SHARD], F32, tag="uTs")
            uY_sb = sb.tile([128, SHARD], F32, tag="uYs")
            nc.scalar.activation(uT_sb[:], u_pT[:], RELU, bias=b1_sb[:, 0:1])
            nc.scalar.activation(uY_sb[:], u_pY[:], RELU, bias=b1_sb[:, 1:2])

            # e = w2_T . u_T + w2_Y . u_Y + (b2_T + b2_Y), both heads into one psum
            e_p = ps.tile([1, SHARD], F32, tag="e")
            nc.tensor.matmul(e_p[:], w2_sb[:, 0:1], uT_sb[:], start=True, stop=False)
            nc.tensor.matmul(e_p[:], w2_sb[:, 1:2], uY_sb[:], start=False, stop=True)

            out_sb = sb.tile([1, SHARD], F32, tag="o")
            nc.scalar.activation(out_sb[:], e_p[:], IDENT, bias=b1_sb[0:1, 2:3])
            nc.sync.dma_start(out[:], out_sb[:])

    nc.compile()
    return nc


def _get_nc():
    global _NC_CACHE
    if _NC_CACHE is None:
        _NC_CACHE = _build_nc()
    return _NC_CACHE


def kernel(**inputs: np.ndarray) -> np.ndarray:
    global LAST_RESULT
    x = np.asarray(inputs["x"], np.float32)
    t = np.asarray(inputs["t"], np.float32)
    y = np.asarray(inputs["y"], np.float32)
    B_param = np.asarray(inputs["B_param"], np.float32)
    fc_in_w = np.asarray(inputs["fc_in_w"], np.float32)
    fc_in_b = np.asarray(inputs["fc_in_b"], np.float32)
    eT_w1 = np.asarray(inputs["eT_w1"], np.float32)
    eT_b1 = np.asarray(inputs["eT_b1"], np.float32)
    eT_w2 = np.asarray(inputs["eT_w2"], np.float32)
    eT_b2 = np.asarray(inputs["eT_b2"], np.float32)
    eY_w1 = np.asarray(inputs["eY_w1"], np.float32)
    eY_b1 = np.asarray(inputs["eY_b1"], np.float32)
    eY_w2 = np.asarray(inputs["eY_w2"], np.float32)
    eY_b2 = np.asarray(inputs["eY_b2"], np.float32)

    # collapse the two message-passing layers to per-node scalars
    n = B_param.shape[0]
    mask = np.ones((n, n), np.float32)
    mask[-1, :D_X] = 0.0
    np.fill_diagonal(mask, 0.0)
    A = mask / (1.0 + np.exp(-B_param))
    c = 1.0 + A.sum(axis=1) / n
    g = c + (A @ c) / n
    gT, gY = np.float32(g[n - 2]), np.float32(g[n - 1])

    w_in_arr = np.zeros((KPAD, HID), np.float32)
    w_in_arr[:D_IN] = fc_in_w.T
    w1g_arr = np.ascontiguousarray(
        np.concatenate([gT * eT_w1.T, gY * eY_w1.T], axis=1)
        .reshape(2, 128, 2 * MLP_HID)
    )
    b_in_arr = np.ascontiguousarray(fc_in_b.reshape(2, 128).T)
    b1_arr = np.zeros((128, 3), np.float32)
    b1_arr[:, 0] = eT_b1
    b1_arr[:, 1] = eY_b1
    b1_arr[0, 2] = eT_b2[0] + eY_b2[0]
    w2_arr = np.ascontiguousarray(np.stack([eT_w2[0], eY_w2[0]], axis=1))

    z = np.concatenate([x, t, y], axis=1)  # [BATCH, 102]
    in_maps = []
    for i in range(N_CORES):
        zT_arr = np.zeros((KPAD, SHARD), np.float32)
        zT_arr[:D_IN] = z[i * SHARD:(i + 1) * SHARD].T
        in_maps.append({
            "zT": zT_arr, "w_in": w_in_arr, "w1g": w1g_arr,
            "b_in": b_in_arr, "b1": b1_arr, "w2": w2_arr,
        })

    nc = _get_nc()
    LAST_RESULT = run_bass_kernel_spmd(nc, in_maps, list(range(N_CORES)))
    return np.concatenate(
        [r["out"].reshape(SHARD) for r in LAST_RESULT.results]
    ).astype(np.float32)



# revision 9
# speedup vs baseline: 1.1160x; 1.1160x over previous
"""Trainium2 Bass kernel for nn_GNN_EBM (gnn_message_passing).

Math: the reference broadcasts one shared feature vector h0[b,:] to all
d_nodes graph nodes before message passing, and the adjacency
A = sigmoid(B_param) * mask is elementwise non-negative.  Hence

  conv1:  relu(h0*(1 + rowsum(A)_i/N))      = c_i * relu(h0)   (c_i > 0)
  conv2:  relu(r*(c_i + (A@c)_i/N))         = g_i * r          (r >= 0, g_i > 0)

so the whole GNN collapses to e = MLP_T(g_T * r) + MLP_Y(g_Y * r) with
r = relu(z @ fc_in_w.T + fc_in_b), and the scalars g_T, g_Y fold into the
MLP first-layer weights.  The device kernel is a fused 3-layer MLP over the
batch, data-parallel across 8 cores (256 rows/core), activations kept
transposed ([feature, batch]) so matmul outputs chain without transposes.

Device-side optimizations vs the first working version:
  - all matmul operands in bf16 (single-pass PE matmuls instead of the
    4-cycles/row fp32 decomposition; half the HBM traffic)
  - exactly two input DMAs, issued in parallel from the two HWDGE engines
    (sync + scalar) instead of seven serialized ~600ns DIRECT2D issues
  - contraction dim kept at 102 (no zero-padding to 128, no memsets)
  - bias+relu fused into one tensor_scalar per layer on the vector
    engine; no scalar-engine activations, so the 1.3us ACT_TABLE_LOAD
    disappears entirely
"""

import sys

sys.path.insert(0, "/opt/trn_rl_repo")

import numpy as np
import ml_dtypes

import concourse.bacc as bacc
import concourse.mybir as mybir
import concourse.tile as tile
from concourse.bass_utils import run_bass_kernel_spmd


def _ensure_ntff_hook():
    # bass_utils' trace path imports antenv.axon_hooks, which some agent
    # images lack; register the ctypes-based hook ourselves so BASS_TRACE=1
    # yields an NTFF profile instead of an ImportError.
    try:
        import antenv.axon_hooks  # noqa: F401
        return
    except ImportError:
        pass
    import types

    import antenv

    mod = types.ModuleType("antenv.axon_hooks")
    holder = {"hook": None}
    mod.set_axon_ntff_profile_hook = lambda h: holder.__setitem__("hook", h)
    mod.get_axon_ntff_profile_hook = lambda: holder["hook"]
    sys.modules["antenv.axon_hooks"] = mod
    antenv.axon_hooks = mod
    try:
        from trn_agent_boot.trn_boot import _ntff_profile_via_ctypes

        hook = _ntff_profile_via_ctypes("/opt/axon/libaxon_pjrt.so")
        if hook is not None:
            mod.set_axon_ntff_profile_hook(hook)
    except Exception:
        pass


_ensure_ntff_hook()

N_CORES = 8
BATCH = 2048
D_X = 100
D_IN = D_X + 2             # x + t + y = 102
HID = 256
MLP_HID = 128
SHARD = BATCH // N_CORES   # 256

F32 = mybir.dt.float32
BF16 = mybir.dt.bfloat16
BF16_NP = ml_dtypes.bfloat16

_NC_CACHE = None
LAST_RESULT = None         # BassKernelResults of the most recent run


def _build_nc():
    nc = bacc.Bacc("TRN2", target_bir_lowering=False, debug=False,
                   num_devices=N_CORES)

    # za: [102, 512] = [ zT (cols 0:256) | fc_in_w.T (cols 256:512) ]
    # wb: [128, 514] = [ w1 k-chunk0 (0:256, cols T|Y) | w1 k-chunk1
    #     (256:512) | w2T | w2Y ]
    # bb: [128, 6] fp32 = [ b_in0 | b_in1 | b1T | b1Y | b2 (row 0) | pad ]
    za_d = nc.dram_tensor("za", [D_IN, 512], BF16, kind="ExternalInput")
    wb_d = nc.dram_tensor("wb", [128, 514], BF16, kind="ExternalInput")
    bb_d = nc.dram_tensor("bb", [128, 6], F32, kind="ExternalInput")
    out_d = nc.dram_tensor("out", [1, SHARD], F32, kind="ExternalOutput")

    ADD = mybir.AluOpType.add
    MAX = mybir.AluOpType.max

    with tile.TileContext(nc) as tc:
        with (
            tc.tile_pool(name="sb", bufs=1) as sb,
            tc.tile_pool(name="ps", bufs=1, space="PSUM") as ps,
        ):
            za = sb.tile([D_IN, 512], BF16, tag="za")
            wb = sb.tile([128, 514], BF16, tag="wb")
            bb = sb.tile([128, 6], F32, tag="bb")
            nc.sync.dma_start(za[:], za_d[:])
            nc.scalar.dma_start(wb[:], wb_d[:])
            nc.sync.dma_start(bb[:], bb_d[:])

            # h^T = fc_in_w @ z^T, feature chunks of 128 on partitions
            h0 = ps.tile([128, SHARD], F32, tag="h0")
            h1 = ps.tile([128, SHARD], F32, tag="h1")
            nc.tensor.matmul(h0[:], za[:, 256:384], za[:, 0:256])
            nc.tensor.matmul(h1[:], za[:, 384:512], za[:, 0:256])

            # r = relu(h + fc_in_b): bias+relu in one vector op per chunk
            r0 = sb.tile([128, SHARD], BF16, tag="r0")
            r1 = sb.tile([128, SHARD], BF16, tag="r1")
            nc.vector.tensor_scalar(r0[:], h0[:], bb[:, 0:1], 0.0, ADD, MAX)
            nc.vector.tensor_scalar(r1[:], h1[:], bb[:, 1:2], 0.0, ADD, MAX)

            # u_head^T = (g_head * w1_head) @ r^T, two k-chunks accumulated
            uT = ps.tile([128, SHARD], F32, tag="uT")
            uY = ps.tile([128, SHARD], F32, tag="uY")
            nc.tensor.matmul(uT[:], wb[:, 0:128], r0[:], start=True, stop=False)
            nc.tensor.matmul(uT[:], wb[:, 256:384], r1[:], start=False, stop=True)
            nc.tensor.matmul(uY[:], wb[:, 128:256], r0[:], start=True, stop=False)
            nc.tensor.matmul(uY[:], wb[:, 384:512], r1[:], start=False, stop=True)

            sT = sb.tile([128, SHARD], BF16, tag="sT")
            sY = sb.tile([128, SHARD], BF16, tag="sY")
            nc.vector.tensor_scalar(sT[:], uT[:], bb[:, 2:3], 0.0, ADD, MAX)
            nc.vector.tensor_scalar(sY[:], uY[:], bb[:, 3:4], 0.0, ADD, MAX)

            # e = w2_T . u_T + w2_Y . u_Y + (b2_T + b2_Y)
            e = ps.tile([1, SHARD], F32, tag="e")
            nc.tensor.matmul(e[:], wb[:, 512:513], sT[:], start=True, stop=False)
            nc.tensor.matmul(e[:], wb[:, 513:514], sY[:], start=False, stop=True)

            o = sb.tile([1, SHARD], F32, tag="o")
            nc.vector.tensor_scalar(o[:], e[:], bb[0:1, 4:5], None, ADD)
            nc.sync.dma_start(out_d[:], o[:])

    nc.compile()
    return nc


def _get_nc():
    global _NC_CACHE
    if _NC_CACHE is None:
        _NC_CACHE = _build_nc()
    return _NC_CACHE


def kernel(**inputs: np.ndarray) -> np.ndarray:
    global LAST_RESULT
    x = np.asarray(inputs["x"], np.float32)
    t = np.asarray(inputs["t"], np.float32)
    y = np.asarray(inputs["y"], np.float32)
    B_param = np.asarray(inputs["B_param"], np.float32)
    fc_in_w = np.asarray(inputs["fc_in_w"], np.float32)
    fc_in_b = np.asarray(inputs["fc_in_b"], np.float32)
    eT_w1 = np.asarray(inputs["eT_w1"], np.float32)
    eT_b1 = np.asarray(inputs["eT_b1"], np.float32)
    eT_w2 = np.asarray(inputs["eT_w2"], np.float32)
    eT_b2 = np.asarray(inputs["eT_b2"], np.float32)
    eY_w1 = np.asarray(inputs["eY_w1"], np.float32)
    eY_b1 = np.asarray(inputs["eY_b1"], np.float32)
    eY_w2 = np.asarray(inputs["eY_w2"], np.float32)
    eY_b2 = np.asarray(inputs["eY_b2"], np.float32)

    # collapse the two message-passing layers to per-node scalars
    n = B_param.shape[0]
    mask = np.ones((n, n), np.float32)
    mask[-1, :D_X] = 0.0
    np.fill_diagonal(mask, 0.0)
    A = mask / (1.0 + np.exp(-B_param))
    c = 1.0 + A.sum(axis=1) / n
    g = c + (A @ c) / n
    gT, gY = np.float32(g[n - 2]), np.float32(g[n - 1])

    w1cat = np.concatenate([gT * eT_w1.T, gY * eY_w1.T], axis=1)  # [256, 256]
    wb_arr = np.zeros((128, 514), BF16_NP)
    wb_arr[:, 0:256] = w1cat[0:128].astype(BF16_NP)
    wb_arr[:, 256:512] = w1cat[128:256].astype(BF16_NP)
    wb_arr[:, 512] = eT_w2[0].astype(BF16_NP)
    wb_arr[:, 513] = eY_w2[0].astype(BF16_NP)

    bb_arr = np.zeros((128, 6), np.float32)
    bb_arr[:, 0] = fc_in_b[0:128]
    bb_arr[:, 1] = fc_in_b[128:256]
    bb_arr[:, 2] = eT_b1
    bb_arr[:, 3] = eY_b1
    bb_arr[0, 4] = eT_b2[0] + eY_b2[0]

    z = np.concatenate([x, t, y], axis=1)  # [BATCH, 102]
    w_in_bf = np.ascontiguousarray(fc_in_w.T).astype(BF16_NP)  # [102, 256]
    in_maps = []
    for i in range(N_CORES):
        za_arr = np.empty((D_IN, 512), BF16_NP)
        za_arr[:, 0:256] = z[i * SHARD:(i + 1) * SHARD].T.astype(BF16_NP)
        za_arr[:, 256:512] = w_in_bf
        in_maps.append({"za": za_arr, "wb": wb_arr, "bb": bb_arr})

    nc = _get_nc()
    LAST_RESULT = run_bass_kernel_spmd(nc, in_maps, list(range(N_CORES)))
    return np.concatenate(
        [r["out"].reshape(SHARD) for r in LAST_RESULT.results]
    ).astype(np.float32)


# revision 10
# speedup vs baseline: 1.2868x; 1.1530x over previous
"""Trainium2 Bass kernel for nn_GNN_EBM (gnn_message_passing).

Math: the reference broadcasts one shared feature vector h0[b,:] to all
d_nodes graph nodes before message passing, and the adjacency
A = sigmoid(B_param) * mask is elementwise non-negative.  Hence

  conv1:  relu(h0*(1 + rowsum(A)_i/N))      = c_i * relu(h0)   (c_i > 0)
  conv2:  relu(r*(c_i + (A@c)_i/N))         = g_i * r          (r >= 0, g_i > 0)

so the whole GNN collapses to e = MLP_T(g_T * r) + MLP_Y(g_Y * r) with
r = relu(z @ fc_in_w.T + fc_in_b), and the scalars g_T, g_Y fold into the
MLP first-layer weights.  The device kernel is a fused 3-layer MLP over the
batch, data-parallel across 8 cores (256 rows/core), activations kept
transposed ([feature, batch]) so matmul outputs chain without transposes.

Device-side schedule (all latency-bound; per-op fixed costs dominate):
  - all matmul operands bf16 (single-pass PE matmuls, half the DMA bytes)
  - fc_in bias folded into the layer-1 matmul via a ones row (k=103), so
    the layer-1 relus are pure max-with-0 and need no bias operand
  - the critical za load is row-split across the two HWDGE engines
    (sync + scalar) so both descriptor streams run in parallel
  - a dummy activation at body start preloads the scalar engine's
    ACT table during the DMA arm window; activations then alternate
    scalar/vector so consecutive relu pairs overlap
"""

import sys

sys.path.insert(0, "/opt/trn_rl_repo")

import numpy as np
import ml_dtypes

import concourse.bacc as bacc
import concourse.mybir as mybir
import concourse.tile as tile
from concourse.bass_utils import run_bass_kernel_spmd


def _ensure_ntff_hook():
    # bass_utils' trace path imports antenv.axon_hooks, which some agent
    # images lack; register the ctypes-based hook ourselves so BASS_TRACE=1
    # yields an NTFF profile instead of an ImportError.
    try:
        import antenv.axon_hooks  # noqa: F401
        return
    except ImportError:
        pass
    import types

    import antenv

    mod = types.ModuleType("antenv.axon_hooks")
    holder = {"hook": None}
    mod.set_axon_ntff_profile_hook = lambda h: holder.__setitem__("hook", h)
    mod.get_axon_ntff_profile_hook = lambda: holder["hook"]
    sys.modules["antenv.axon_hooks"] = mod
    antenv.axon_hooks = mod
    try:
        from trn_agent_boot.trn_boot import _ntff_profile_via_ctypes

        hook = _ntff_profile_via_ctypes("/opt/axon/libaxon_pjrt.so")
        if hook is not None:
            mod.set_axon_ntff_profile_hook(hook)
    except Exception:
        pass


_ensure_ntff_hook()

N_CORES = 8
BATCH = 2048
D_X = 100
D_IN = D_X + 2             # x + t + y = 102
KD = D_IN + 1              # + ones row for the fc_in bias = 103
HID = 256
MLP_HID = 128
SHARD = BATCH // N_CORES   # 256

F32 = mybir.dt.float32
BF16 = mybir.dt.bfloat16
BF16_NP = ml_dtypes.bfloat16

_NC_CACHE = None
LAST_RESULT = None         # BassKernelResults of the most recent run


def _build_nc():
    nc = bacc.Bacc("TRN2", target_bir_lowering=False, debug=False,
                   num_devices=N_CORES)

    # za: [103, 512] = [ zT;ones (cols 0:256) | fc_in_w.T;fc_in_b (256:512) ]
    # wb: [128, 514] = [ w1 k-chunk0 | w1 k-chunk1 | w2T | w2Y ]  (bf16)
    # bb: [128, 4] fp32 = [ b1T | b1Y | b2 (row 0) | pad ]
    za_d = nc.dram_tensor("za", [KD, 512], BF16, kind="ExternalInput")
    wb_d = nc.dram_tensor("wb", [128, 514], BF16, kind="ExternalInput")
    bb_d = nc.dram_tensor("bb", [128, 4], F32, kind="ExternalInput")
    out_d = nc.dram_tensor("out", [1, SHARD], F32, kind="ExternalOutput")

    ADD = mybir.AluOpType.add
    MAX = mybir.AluOpType.max
    RELU = mybir.ActivationFunctionType.Relu
    IDENT = mybir.ActivationFunctionType.Identity

    with tile.TileContext(nc) as tc:
        with (
            tc.tile_pool(name="sb", bufs=1) as sb,
            tc.tile_pool(name="ps", bufs=1, space="PSUM") as ps,
        ):
            za = sb.tile([KD, 512], BF16, tag="za")
            wb = sb.tile([128, 514], BF16, tag="wb")
            bb = sb.tile([128, 4], F32, tag="bb")
            dum = sb.tile([128, 1], F32, tag="dum")

            # critical z/w_in load split across both HWDGE engines
            nc.sync.dma_start(za[0:52, :], za_d[0:52, :])
            nc.scalar.dma_start(za[52:KD, :], za_d[52:KD, :])
            nc.sync.dma_start(bb[:], bb_d[:])
            nc.scalar.dma_start(wb[:], wb_d[:])
            # dummy activation: forces the ACT table load to happen now,
            # hidden under the DMA arm latency
            nc.scalar.activation(dum[:], nc.const_aps.aps[(F32, 0.0)], RELU)

            # h^T = [fc_in_w | fc_in_b] @ [z^T ; 1], feature chunks of 128
            h0 = ps.tile([128, SHARD], F32, tag="h0")
            h1 = ps.tile([128, SHARD], F32, tag="h1")
            nc.tensor.matmul(h0[:], za[:, 256:384], za[:, 0:256])
            nc.tensor.matmul(h1[:], za[:, 384:512], za[:, 0:256])

            # r = relu(h), bias already folded in; split across vector/scalar
            r0 = sb.tile([128, SHARD], BF16, tag="r0")
            r1 = sb.tile([128, SHARD], BF16, tag="r1")
            nc.vector.tensor_scalar(r0[:], h0[:], 0.0, None, MAX)
            nc.scalar.activation(r1[:], h1[:], RELU)

            # u_head^T = (g_head * w1_head) @ r^T, two k-chunks accumulated
            uT = ps.tile([128, SHARD], F32, tag="uT")
            uY = ps.tile([128, SHARD], F32, tag="uY")
            nc.tensor.matmul(uT[:], wb[:, 0:128], r0[:], start=True, stop=False)
            nc.tensor.matmul(uT[:], wb[:, 256:384], r1[:], start=False, stop=True)
            nc.tensor.matmul(uY[:], wb[:, 128:256], r0[:], start=True, stop=False)
            nc.tensor.matmul(uY[:], wb[:, 384:512], r1[:], start=False, stop=True)

            # s = relu(u + b1), bias+relu fused, scalar/vector in parallel
            sT = sb.tile([128, SHARD], BF16, tag="sT")
            sY = sb.tile([128, SHARD], BF16, tag="sY")
            nc.scalar.activation(sT[:], uT[:], RELU, bias=bb[:, 0:1])
            nc.vector.tensor_scalar(sY[:], uY[:], bb[:, 1:2], 0.0, ADD, MAX)

            # e = w2_T . s_T + w2_Y . s_Y + (b2_T + b2_Y)
            e = ps.tile([1, SHARD], F32, tag="e")
            nc.tensor.matmul(e[:], wb[:, 512:513], sT[:], start=True, stop=False)
            nc.tensor.matmul(e[:], wb[:, 513:514], sY[:], start=False, stop=True)

            o = sb.tile([1, SHARD], F32, tag="o")
            nc.scalar.activation(o[:], e[:], IDENT, bias=bb[0:1, 2:3])
            nc.sync.dma_start(out_d[:], o[:])

    nc.compile()
    return nc


def _get_nc():
    global _NC_CACHE
    if _NC_CACHE is None:
        _NC_CACHE = _build_nc()
    return _NC_CACHE


def kernel(**inputs: np.ndarray) -> np.ndarray:
    global LAST_RESULT
    x = np.asarray(inputs["x"], np.float32)
    t = np.asarray(inputs["t"], np.float32)
    y = np.asarray(inputs["y"], np.float32)
    B_param = np.asarray(inputs["B_param"], np.float32)
    fc_in_w = np.asarray(inputs["fc_in_w"], np.float32)
    fc_in_b = np.asarray(inputs["fc_in_b"], np.float32)
    eT_w1 = np.asarray(inputs["eT_w1"], np.float32)
    eT_b1 = np.asarray(inputs["eT_b1"], np.float32)
    eT_w2 = np.asarray(inputs["eT_w2"], np.float32)
    eT_b2 = np.asarray(inputs["eT_b2"], np.float32)
    eY_w1 = np.asarray(inputs["eY_w1"], np.float32)
    eY_b1 = np.asarray(inputs["eY_b1"], np.float32)
    eY_w2 = np.asarray(inputs["eY_w2"], np.float32)
    eY_b2 = np.asarray(inputs["eY_b2"], np.float32)

    # collapse the two message-passing layers to per-node scalars
    n = B_param.shape[0]
    mask = np.ones((n, n), np.float32)
    mask[-1, :D_X] = 0.0
    np.fill_diagonal(mask, 0.0)
    A = mask / (1.0 + np.exp(-B_param))
    c = 1.0 + A.sum(axis=1) / n
    g = c + (A @ c) / n
    gT, gY = np.float32(g[n - 2]), np.float32(g[n - 1])

    w1cat = np.concatenate([gT * eT_w1.T, gY * eY_w1.T], axis=1)  # [256, 256]
    wb_arr = np.zeros((128, 514), BF16_NP)
    wb_arr[:, 0:256] = w1cat[0:128].astype(BF16_NP)
    wb_arr[:, 256:512] = w1cat[128:256].astype(BF16_NP)
    wb_arr[:, 512] = eT_w2[0].astype(BF16_NP)
    wb_arr[:, 513] = eY_w2[0].astype(BF16_NP)

    bb_arr = np.zeros((128, 4), np.float32)
    bb_arr[:, 0] = eT_b1
    bb_arr[:, 1] = eY_b1
    bb_arr[0, 2] = eT_b2[0] + eY_b2[0]

    z = np.concatenate([x, t, y], axis=1)  # [BATCH, 102]
    wcol = np.empty((KD, 256), np.float32)  # [fc_in_w.T ; fc_in_b]
    wcol[0:D_IN] = fc_in_w.T
    wcol[D_IN] = fc_in_b
    wcol_bf = wcol.astype(BF16_NP)
    in_maps = []
    for i in range(N_CORES):
        za_arr = np.empty((KD, 512), BF16_NP)
        za_arr[0:D_IN, 0:256] = z[i * SHARD:(i + 1) * SHARD].T.astype(BF16_NP)
        za_arr[D_IN, 0:256] = BF16_NP(1.0)
        za_arr[:, 256:512] = wcol_bf
        in_maps.append({"za": za_arr, "wb": wb_arr, "bb": bb_arr})

    nc = _get_nc()
    LAST_RESULT = run_bass_kernel_spmd(nc, in_maps, list(range(N_CORES)))
    return np.concatenate(
        [r["out"].reshape(SHARD) for r in LAST_RESULT.results]
    ).astype(np.float32)


# revision 11
# speedup vs baseline: 1.3047x; 1.0140x over previous
"""Trainium2 Bass kernel for nn_GNN_EBM (gnn_message_passing).

Math: the reference broadcasts one shared feature vector h0[b,:] to all
d_nodes graph nodes before message passing, and the adjacency
A = sigmoid(B_param) * mask is elementwise non-negative.  Hence

  conv1:  relu(h0*(1 + rowsum(A)_i/N))      = c_i * relu(h0)   (c_i > 0)
  conv2:  relu(r*(c_i + (A@c)_i/N))         = g_i * r          (r >= 0, g_i > 0)

so the whole GNN collapses to e = MLP_T(g_T * r) + MLP_Y(g_Y * r) with
r = relu(z @ fc_in_w.T + fc_in_b), and the scalars g_T, g_Y fold into the
MLP first-layer weights.  The device kernel is a fused 3-layer MLP over the
batch, data-parallel across 8 cores (256 rows/core), activations kept
transposed ([feature, batch]) so matmul outputs chain without transposes.

Device-side schedule (all latency-bound; per-op fixed costs dominate):
  - all matmul operands bf16 (single-pass PE matmuls, half the DMA bytes)
  - fc_in bias folded into the layer-1 matmul via a ones row (k=103), so
    the layer-1 relus are pure max-with-0 and need no bias operand
  - the critical za load is row-split across the two HWDGE engines
    (sync + scalar) so both descriptor streams run in parallel
  - a dummy activation at body start preloads the scalar engine's
    ACT table during the DMA arm window; activations then alternate
    scalar/vector so consecutive relu pairs overlap
"""

import sys

sys.path.insert(0, "/opt/trn_rl_repo")

import numpy as np
import ml_dtypes

import concourse.bacc as bacc
import concourse.mybir as mybir
import concourse.tile as tile
from concourse.bass_utils import run_bass_kernel_spmd


def _ensure_ntff_hook():
    # bass_utils' trace path imports antenv.axon_hooks, which some agent
    # images lack; register the ctypes-based hook ourselves so BASS_TRACE=1
    # yields an NTFF profile instead of an ImportError.
    try:
        import antenv.axon_hooks  # noqa: F401
        return
    except ImportError:
        pass
    import types

    import antenv

    mod = types.ModuleType("antenv.axon_hooks")
    holder = {"hook": None}
    mod.set_axon_ntff_profile_hook = lambda h: holder.__setitem__("hook", h)
    mod.get_axon_ntff_profile_hook = lambda: holder["hook"]
    sys.modules["antenv.axon_hooks"] = mod
    antenv.axon_hooks = mod
    try:
        from trn_agent_boot.trn_boot import _ntff_profile_via_ctypes

        hook = _ntff_profile_via_ctypes("/opt/axon/libaxon_pjrt.so")
        if hook is not None:
            mod.set_axon_ntff_profile_hook(hook)
    except Exception:
        pass


_ensure_ntff_hook()

N_CORES = 8
BATCH = 2048
D_X = 100
D_IN = D_X + 2             # x + t + y = 102
KD = D_IN + 1              # + ones row for the fc_in bias = 103
HID = 256
MLP_HID = 128
SHARD = BATCH // N_CORES   # 256

F32 = mybir.dt.float32
BF16 = mybir.dt.bfloat16
BF16_NP = ml_dtypes.bfloat16

_NC_CACHE = None
LAST_RESULT = None         # BassKernelResults of the most recent run


def _build_nc():
    nc = bacc.Bacc("TRN2", target_bir_lowering=False, debug=False,
                   num_devices=N_CORES)

    # za: [103, 512] = [ zT;ones (cols 0:256) | fc_in_w.T;fc_in_b (256:512) ]
    # wb: [128, 514] = [ w1 k-chunk0 | w1 k-chunk1 | w2T | w2Y ]  (bf16)
    # bb: [128, 4] fp32 = [ b1T | b1Y | b2 (row 0) | pad ]
    za_d = nc.dram_tensor("za", [KD, 512], BF16, kind="ExternalInput")
    wb_d = nc.dram_tensor("wb", [128, 514], BF16, kind="ExternalInput")
    bb_d = nc.dram_tensor("bb", [128, 4], F32, kind="ExternalInput")
    out_d = nc.dram_tensor("out", [1, SHARD], F32, kind="ExternalOutput")

    ADD = mybir.AluOpType.add
    MAX = mybir.AluOpType.max
    RELU = mybir.ActivationFunctionType.Relu
    IDENT = mybir.ActivationFunctionType.Identity

    with tile.TileContext(nc) as tc:
        with (
            tc.tile_pool(name="sb", bufs=1) as sb,
            tc.tile_pool(name="ps", bufs=1, space="PSUM") as ps,
        ):
            za = sb.tile([KD, 512], BF16, tag="za")
            wb = sb.tile([128, 514], BF16, tag="wb")
            bb = sb.tile([128, 4], F32, tag="bb")
            dum = sb.tile([128, 1], F32, tag="dum")

            # critical z/w_in load split across both HWDGE engines.
            # NOTE: HWDGE completion sems count per-engine, so anything
            # queued after za on the same engine delays za's waiters;
            # sync's queue carries ONLY za_lo (then the output much later),
            # scalar's queue carries za_hi -> wb -> bb in need order.
            nc.sync.dma_start(za[0:52, :], za_d[0:52, :])
            nc.scalar.dma_start(za[52:KD, :], za_d[52:KD, :])
            nc.scalar.dma_start(wb[:], wb_d[:])
            nc.scalar.dma_start(bb[:], bb_d[:])
            # dummy activation: forces the ACT table load to happen now,
            # hidden under the DMA arm latency
            nc.scalar.activation(dum[:], nc.const_aps.aps[(F32, 0.0)], RELU)

            # h^T = [fc_in_w | fc_in_b] @ [z^T ; 1], feature chunks of 128
            h0 = ps.tile([128, SHARD], F32, tag="h0")
            h1 = ps.tile([128, SHARD], F32, tag="h1")
            nc.tensor.matmul(h0[:], za[:, 256:384], za[:, 0:256])
            nc.tensor.matmul(h1[:], za[:, 384:512], za[:, 0:256])

            # r = relu(h), bias already folded in; split across vector/scalar
            r0 = sb.tile([128, SHARD], BF16, tag="r0")
            r1 = sb.tile([128, SHARD], BF16, tag="r1")
            nc.vector.tensor_scalar(r0[:], h0[:], 0.0, None, MAX)
            nc.scalar.activation(r1[:], h1[:], RELU)

            # u_head^T = (g_head * w1_head) @ r^T, two k-chunks accumulated
            uT = ps.tile([128, SHARD], F32, tag="uT")
            uY = ps.tile([128, SHARD], F32, tag="uY")
            nc.tensor.matmul(uT[:], wb[:, 0:128], r0[:], start=True, stop=False)
            nc.tensor.matmul(uT[:], wb[:, 256:384], r1[:], start=False, stop=True)
            nc.tensor.matmul(uY[:], wb[:, 128:256], r0[:], start=True, stop=False)
            nc.tensor.matmul(uY[:], wb[:, 384:512], r1[:], start=False, stop=True)

            # s = relu(u + b1), bias+relu fused, scalar/vector in parallel
            sT = sb.tile([128, SHARD], BF16, tag="sT")
            sY = sb.tile([128, SHARD], BF16, tag="sY")
            nc.scalar.activation(sT[:], uT[:], RELU, bias=bb[:, 0:1])
            nc.vector.tensor_scalar(sY[:], uY[:], bb[:, 1:2], 0.0, ADD, MAX)

            # e = w2_T . s_T + w2_Y . s_Y + (b2_T + b2_Y)
            e = ps.tile([1, SHARD], F32, tag="e")
            nc.tensor.matmul(e[:], wb[:, 512:513], sT[:], start=True, stop=False)
            nc.tensor.matmul(e[:], wb[:, 513:514], sY[:], start=False, stop=True)

            o = sb.tile([1, SHARD], F32, tag="o")
            nc.scalar.activation(o[:], e[:], IDENT, bias=bb[0:1, 2:3])
            nc.sync.dma_start(out_d[:], o[:])

    nc.compile()
    return nc


def _get_nc():
    global _NC_CACHE
    if _NC_CACHE is None:
        _NC_CACHE = _build_nc()
    return _NC_CACHE


def kernel(**inputs: np.ndarray) -> np.ndarray:
    global LAST_RESULT
    x = np.asarray(inputs["x"], np.float32)
    t = np.asarray(inputs["t"], np.float32)
    y = np.asarray(inputs["y"], np.float32)
    B_param = np.asarray(inputs["B_param"], np.float32)
    fc_in_w = np.asarray(inputs["fc_in_w"], np.float32)
    fc_in_b = np.asarray(inputs["fc_in_b"], np.float32)
    eT_w1 = np.asarray(inputs["eT_w1"], np.float32)
    eT_b1 = np.asarray(inputs["eT_b1"], np.float32)
    eT_w2 = np.asarray(inputs["eT_w2"], np.float32)
    eT_b2 = np.asarray(inputs["eT_b2"], np.float32)
    eY_w1 = np.asarray(inputs["eY_w1"], np.float32)
    eY_b1 = np.asarray(inputs["eY_b1"], np.float32)
    eY_w2 = np.asarray(inputs["eY_w2"], np.float32)
    eY_b2 = np.asarray(inputs["eY_b2"], np.float32)

    # collapse the two message-passing layers to per-node scalars
    n = B_param.shape[0]
    mask = np.ones((n, n), np.float32)
    mask[-1, :D_X] = 0.0
    np.fill_diagonal(mask, 0.0)
    A = mask / (1.0 + np.exp(-B_param))
    c = 1.0 + A.sum(axis=1) / n
    g = c + (A @ c) / n
    gT, gY = np.float32(g[n - 2]), np.float32(g[n - 1])

    w1cat = np.concatenate([gT * eT_w1.T, gY * eY_w1.T], axis=1)  # [256, 256]
    wb_arr = np.zeros((128, 514), BF16_NP)
    wb_arr[:, 0:256] = w1cat[0:128].astype(BF16_NP)
    wb_arr[:, 256:512] = w1cat[128:256].astype(BF16_NP)
    wb_arr[:, 512] = eT_w2[0].astype(BF16_NP)
    wb_arr[:, 513] = eY_w2[0].astype(BF16_NP)

    bb_arr = np.zeros((128, 4), np.float32)
    bb_arr[:, 0] = eT_b1
    bb_arr[:, 1] = eY_b1
    bb_arr[0, 2] = eT_b2[0] + eY_b2[0]

    z = np.concatenate([x, t, y], axis=1)  # [BATCH, 102]
    wcol = np.empty((KD, 256), np.float32)  # [fc_in_w.T ; fc_in_b]
    wcol[0:D_IN] = fc_in_w.T
    wcol[D_IN] = fc_in_b
    wcol_bf = wcol.astype(BF16_NP)
    in_maps = []
    for i in range(N_CORES):
        za_arr = np.empty((KD, 512), BF16_NP)
        za_arr[0:D_IN, 0:256] = z[i * SHARD:(i + 1) * SHARD].T.astype(BF16_NP)
        za_arr[D_IN, 0:256] = BF16_NP(1.0)
        za_arr[:, 256:512] = wcol_bf
        in_maps.append({"za": za_arr, "wb": wb_arr, "bb": bb_arr})

    nc = _get_nc()
    LAST_RESULT = run_bass_kernel_spmd(nc, in_maps, list(range(N_CORES)))
    return np.concatenate(
        [r["out"].reshape(SHARD) for r in LAST_RESULT.results]
    ).astype(np.float32)
